# revision 15
# baseline (speedup 1.0000x reference)
"""Trainium2 Bass kernel for nn_Encoder_50852412785097 (sparse_attention).

Math (validated against the jax reference to ~1e-6):
  Per (b, h):
    Q = X wQ_h, K = X wK_h, V = X wV_h              (X = inputs[b], [S, D])
    e = (Q K^T) / sqrt(D)
    x = causal_softmax(e)          # == softmax(e) * tril, renormalized
    rr = den - cumsum(exp(e_row))  # den = masked row sum of exp(e)
    decay = exp((theta^2/den) * (t_j - t_i) * rr)   # == exp(-theta^2 (1-c) dt)
    u = exp(e * decay)             # unnormalized second softmax
    out_h = ((u @ V) / sum_j u) @ wO_h
  out[b] = sum_h out_h

Sharding: 16 (b, h) pairs over 8 cores -> core c handles b = c//4,
heads {2*(c%4), 2*(c%4)+1}. Weights replicated; host sums the 4 partial
outputs per batch.

SPMD program per core: flash-style over 16 row-tiles of 128 rows, only the
causal-active W = 128*(ti+1) columns are computed. cumsum via DVE
tensor_tensor_scan; softmax sums fused into ACT exp via accum_out; AV via
PE 128x128 transposes of u.
"""

import os
import sys

import numpy as np

B, S, H, D = 2, 2048, 8, 64
P = 128
NT = S // P  # 16 row tiles
NH = 2  # heads per core
NCORES = 8
MASK_VAL = -1e30


def _import_concourse():
    try:
        import concourse.bass  # noqa: F401
    except ImportError:
        for p in ("/opt/trn_rl_repo", "/root/.axon_site/_ro/trn_rl_repo"):
            if os.path.isdir(p) and p not in sys.path:
                sys.path.insert(0, p)
        import concourse.bass  # noqa: F401


def build_nc(use_dma_transpose=True, sa_gpsimd=True):
    """Build the SPMD single-core program (same on all 8 cores)."""
    _import_concourse()
    import concourse.bass as bass
    import concourse.bacc as bacc
    from concourse import mybir
    from concourse.tile import TileContext

    f32 = mybir.dt.float32
    bf16 = mybir.dt.bfloat16
    Alu = mybir.AluOpType
    Act = mybir.ActivationFunctionType

    # Bacc (not raw Bass): its finalize() runs move_matmul_waits_to_ldweights
    # + generate_event_semaphores, legalizing the 1-sync-wait-per-instruction
    # hardware constraint that walrus enforces.
    nc = bacc.Bacc("TRN2", target_bir_lowering=False, debug=False)

    # --- external I/O (per core) ---
    xT_h = nc.dram_tensor("xT", [D, S], f32, kind="ExternalInput")     # X^T
    tsj_h = nc.dram_tensor("tsj", [1, S], f32, kind="ExternalInput")   # t_j row
    tsi_h = nc.dram_tensor("tsi", [P, NT], f32, kind="ExternalInput")  # t_i cols
    wq_h = nc.dram_tensor("wq", [D, NH * D], f32, kind="ExternalInput")
    wk_h = nc.dram_tensor("wk", [D, NH * D], f32, kind="ExternalInput")
    wv_h = nc.dram_tensor("wv", [D, NH * D], f32, kind="ExternalInput")
    wo_h = nc.dram_tensor("wo", [D, NH * D], f32, kind="ExternalInput")
    th_h = nc.dram_tensor("th", [1, 1], f32, kind="ExternalInput")
    y_h = nc.dram_tensor("y", [S, D], f32, kind="ExternalOutput")

    # --- NEFF-embedded constants ---
    mask_np = np.triu(np.ones((P, P), np.float32), k=1) * np.float32(MASK_VAL)
    mask_dram = nc.inline_tensor(mask_np, name="maskc")
    ident_dram = nc.inline_tensor(np.eye(P, dtype=np.float32).astype(
        np.dtype("bfloat16") if False else np.float32), name="identc")

    with TileContext(nc) as tc:
        from contextlib import ExitStack

        with ExitStack() as ctx:
            consts = ctx.enter_context(tc.tile_pool(name="consts", bufs=1))

            # DMA inputs into staging tiles, then stage through a single
            # compute engine so downstream consumers wait on ONE semaphore.
            # (walrus allows only one sync-wait on PE LDWEIGHTS: every
            # PE-consumed tile must be DVE-produced; DVE-consumed multi-DMA
            # tiles go through ACT.)
            def load(shape, handle_ap, via, name, dt=f32):
                stage = consts.tile(shape, f32, tag=f"stg_{name}")
                nc.gpsimd.dma_start(out=stage, in_=handle_ap)
                dst = consts.tile(shape, dt, tag=name)
                via(dst, stage)
                return dst

            # PE-consumed: staged via DVE (order: ident/mask first so PE's
            # DVE-tick already covers them by first use)
            mask = load([P, P], mask_dram[:, :], nc.vector.tensor_copy, "mask")
            identb = load([P, P], ident_dram[:, :], nc.vector.tensor_copy,
                          "identb", dt=bf16)
            xT = load([D, S], xT_h[:, :], nc.vector.tensor_copy, "xT")
            xTb = consts.tile([D, S], bf16, tag="xTb")
            nc.vector.tensor_copy(xTb, xT)
            wq = load([D, NH * D], wq_h[:, :], nc.vector.tensor_copy, "wq", dt=bf16)
            wk = load([D, NH * D], wk_h[:, :], nc.vector.tensor_copy, "wk", dt=bf16)
            wv = load([D, NH * D], wv_h[:, :], nc.vector.tensor_copy, "wv", dt=bf16)
            wo = load([D, NH * D], wo_h[:, :], nc.vector.tensor_copy, "wo", dt=bf16)

            # DVE-consumed (ww op): staged via ACT so the ww instruction's
            # waits collapse onto the ACT semaphore
            tsj_ap = tsj_h[:, :]
            tsj_b = bass.AP(
                tensor=tsj_ap.tensor, offset=tsj_ap.offset,
                ap=[[0, P], list(tsj_ap.ap[-1])],
            )
            tsj = load([P, S], tsj_b, nc.scalar.copy, "tsj")
            tsi = load([P, NT], tsi_h[:, :], nc.scalar.copy, "tsi")

            # theta broadcast -> th2 = theta^2 (DVE-consumed once)
            thb = consts.tile([P, 1], f32)
            th_ap = th_h[:, :]
            th_b = bass.AP(
                tensor=th_ap.tensor, offset=th_ap.offset,
                ap=[[0, P], list(th_ap.ap[-1])],
            )
            nc.gpsimd.dma_start(out=thb, in_=th_b)
            th2 = consts.tile([P, 1], f32)
            nc.vector.tensor_mul(th2, thb, thb)

            ones = consts.tile([P, S], f32)
            nc.vector.memset(ones, 1.0)

            # --- projections: qt (scaled by 1/8), kt: [64, NH*S]; v: [128, NH*NT*64] ---
            qt = consts.tile([D, NH * S], bf16)
            kt = consts.tile([D, NH * S], bf16)
            v = consts.tile([P, NH * NT * D], bf16)
            with tc.tile_pool(name="psetup", bufs=2, space="PSUM") as psetup:
                for h in range(NH):
                    for sc in range(S // 512):
                        pq = psetup.tile([D, 512], f32, tag="pq")
                        nc.tensor.matmul(
                            pq, lhsT=wq[:, h * D:(h + 1) * D],
                            rhs=xTb[:, 512 * sc:512 * (sc + 1)],
                            start=True, stop=True,
                        )
                        nc.scalar.mul(qt[:, h * S + 512 * sc: h * S + 512 * (sc + 1)], pq, 0.125)
                        pk = psetup.tile([D, 512], f32, tag="pk")
                        nc.tensor.matmul(
                            pk, lhsT=wk[:, h * D:(h + 1) * D],
                            rhs=xTb[:, 512 * sc:512 * (sc + 1)],
                            start=True, stop=True,
                        )
                        nc.scalar.copy(kt[:, h * S + 512 * sc: h * S + 512 * (sc + 1)], pk)
                    for st in range(NT):
                        pv = psetup.tile([P, D], f32, tag="pv")
                        nc.tensor.matmul(
                            pv, lhsT=xTb[:, P * st:P * (st + 1)],
                            rhs=wv[:, h * D:(h + 1) * D],
                            start=True, stop=True,
                        )
                        nc.scalar.copy(v[:, (h * NT + st) * D:(h * NT + st + 1) * D], pv)

            # --- main pipeline ---
            work = ctx.enter_context(tc.tile_pool(name="work", bufs=3))
            small = ctx.enter_context(tc.tile_pool(name="small", bufs=6))
            ppe = ctx.enter_context(tc.tile_pool(
                name="ppe", bufs=3 if use_dma_transpose else 2, space="PSUM"))
            ppt = (None if use_dma_transpose else ctx.enter_context(
                tc.tile_pool(name="ppt", bufs=2, space="PSUM")))
            pprT = ctx.enter_context(tc.tile_pool(name="pprT", bufs=2, space="PSUM"))
            ppo = ctx.enter_context(tc.tile_pool(name="ppo", bufs=2, space="PSUM"))

            for ti in range(NT):
                W = P * (ti + 1)
                t_i = tsi[:, ti:ti + 1]
                po_h = []
                rden2_h = []
                for h in range(NH):
                    # scores: es = (Q K^T)/8, diag block gets causal mask added
                    es = work.tile([P, S], f32, tag="es")
                    qrow = qt[:, h * S + P * ti: h * S + P * (ti + 1)]
                    j0 = 0
                    while j0 < W:
                        j1 = min(W, j0 + 512)
                        cols = j1 - j0
                        pe = ppe.tile([P, 512], f32, tag="pe")
                        nc.tensor.matmul(
                            pe[:, :cols], lhsT=qrow,
                            rhs=kt[:, h * S + j0: h * S + j1],
                            start=True, stop=True,
                        )
                        if j1 == W:
                            # single-engine readers per PSUM tile (PE WAR
                            # waits must stay on one semaphore)
                            if cols > P:
                                nc.vector.tensor_copy(es[:, j0:j1 - P], pe[:, :cols - P])
                            nc.vector.tensor_add(
                                es[:, W - P:W], pe[:, cols - P:cols], mask)
                        else:
                            nc.vector.tensor_copy(es[:, j0:j1], pe[:, :cols])
                        j0 = j1

                    # first softmax pieces: ex1 = exp(es), den = row sum
                    ex1 = work.tile([P, S], f32, tag="ex1")
                    den = small.tile([P, 1], f32, tag="den")
                    nc.scalar.activation(ex1[:, :W], es[:, :W], Act.Exp,
                                         accum_out=den)
                    rden = small.tile([P, 1], f32, tag="rden")
                    nc.vector.reciprocal(rden, den)
                    spp = small.tile([P, 1], f32, tag="spp")
                    nc.vector.tensor_mul(spp, th2, rden)

                    # rr = den - cumsum(ex1)  (scan: state=(1*state)-ex1_t)
                    rr = work.tile([P, S], f32, tag="rr")
                    nc.vector.tensor_tensor_scan(
                        rr[:, :W], ones[:, :W], ex1[:, :W], initial=den,
                        op0=Alu.mult, op1=Alu.subtract,
                    )
                    # ww = (t_j - t_i) * rr   (in-place into rr)
                    nc.vector.scalar_tensor_tensor(
                        rr[:, :W], in0=tsj[:, :W], scalar=t_i, in1=rr[:, :W],
                        op0=Alu.subtract, op1=Alu.mult,
                    )
                    # decay = exp(spp * ww); sarr = es * decay (in-place);
                    # u = exp(sarr), den2 = row sum
                    e2 = work.tile([P, S], f32, tag="e2")
                    nc.scalar.activation(e2[:, :W], rr[:, :W], Act.Exp,
                                         scale=spp)
                    (nc.gpsimd if sa_gpsimd else nc.vector).tensor_mul(
                        e2[:, :W], es[:, :W], e2[:, :W])
                    u = work.tile([P, S], bf16, tag="u")
                    den2 = small.tile([P, 1], f32, tag="den2")
                    nc.scalar.activation(u[:, :W], e2[:, :W], Act.Exp,
                                         accum_out=den2)
                    rden2 = small.tile([P, 1], f32, tag="rden2")
                    nc.vector.reciprocal(rden2, den2)
                    rden2_h.append(rden2)

                    # AV: retT[e, i] = sum_j v[j, e] u[i, j]
                    prT = pprT.tile([D, P], f32, tag="prT")
                    njb = ti + 1
                    for g0 in range(0, njb, 4):
                        gn = min(4, njb - g0)
                        uT4 = small.tile([P, 4 * P], bf16, tag="uT4")
                        if use_dma_transpose:
                            for q in range(gn):
                                nc.sync.dma_start_transpose(
                                    uT4[:, q * P:(q + 1) * P],
                                    u[:, (g0 + q) * P:(g0 + q + 1) * P])
                        else:
                            pt = ppt.tile([P, 4 * P], bf16, tag="pt")
                            for q in range(gn):
                                nc.tensor.transpose(
                                    pt[:, q * P:(q + 1) * P],
                                    u[:, (g0 + q) * P:(g0 + q + 1) * P], identb)
                            nc.vector.tensor_copy(uT4[:, :gn * P], pt[:, :gn * P])
                        for q in range(gn):
                            jb = g0 + q
                            nc.tensor.matmul(
                                prT, lhsT=v[:, (h * NT + jb) * D:(h * NT + jb + 1) * D],
                                rhs=uT4[:, q * P:(q + 1) * P],
                                start=(jb == 0), stop=(jb == ti),
                            )
                    rT = small.tile([D, P], bf16, tag="rT")
                    nc.vector.tensor_copy(rT, prT)
                    po = ppo.tile([P, D], f32, tag="po")
                    nc.tensor.matmul(po, lhsT=rT, rhs=wo[:, h * D:(h + 1) * D],
                                     start=True, stop=True)
                    po_h.append(po)

                # y = po0/den2_0 + po1/den2_1 ; DMA out
                t0 = small.tile([P, D], f32, tag="t0")
                nc.vector.tensor_scalar(t0, po_h[0], scalar1=rden2_h[0],
                                        scalar2=None, op0=Alu.mult)
                ys = small.tile([P, D], f32, tag="ys")
                nc.vector.scalar_tensor_tensor(
                    ys, in0=po_h[1], scalar=rden2_h[1], in1=t0,
                    op0=Alu.mult, op1=Alu.add,
                )
                nc.sync.dma_start(out=y_h[P * ti:P * (ti + 1), :], in_=ys)

    if not nc.is_finalized():
        nc.finalize()
    return nc


_NC_CACHE = {}

KERNEL_FLAGS = {}


def _get_nc():
    key = tuple(sorted(KERNEL_FLAGS.items()))
    if key not in _NC_CACHE:
        _NC_CACHE[key] = build_nc(**KERNEL_FLAGS)
    return _NC_CACHE[key]


def make_in_maps(inputs, timestamp, wQ, wK, wV, wO, theta):
    x = np.asarray(inputs, np.float32)
    t = np.asarray(timestamp).astype(np.float32)
    wQ = np.asarray(wQ, np.float32)
    wK = np.asarray(wK, np.float32)
    wV = np.asarray(wV, np.float32)
    wO = np.asarray(wO, np.float32)
    theta = np.asarray(theta, np.float32)

    in_maps = []
    for c in range(NCORES):
        b = c // 4
        h0 = NH * (c % 4)
        in_maps.append({
            "xT": np.ascontiguousarray(x[b].T),
            "tsj": np.ascontiguousarray(t[b][None, :]),
            "tsi": np.ascontiguousarray(t[b].reshape(NT, P).T),
            "wq": np.ascontiguousarray(np.concatenate([wQ[h0], wQ[h0 + 1]], axis=1)),
            "wk": np.ascontiguousarray(np.concatenate([wK[h0], wK[h0 + 1]], axis=1)),
            "wv": np.ascontiguousarray(np.concatenate([wV[h0], wV[h0 + 1]], axis=1)),
            "wo": np.ascontiguousarray(np.concatenate(
                [wO[h0 * D:(h0 + 1) * D], wO[(h0 + 1) * D:(h0 + 2) * D]], axis=1)),
            "th": np.ascontiguousarray(theta.reshape(1, 1)),
        })
    return in_maps


def kernel(inputs, timestamp, wQ, wK, wV, wO, theta, _trace=False, _trace_kwargs=None):
    _import_concourse()
    from concourse.bass_utils import run_bass_kernel_spmd

    nc = _get_nc()
    in_maps = make_in_maps(inputs, timestamp, wQ, wK, wV, wO, theta)
    res = run_bass_kernel_spmd(
        nc, in_maps, list(range(NCORES)),
        trace=_trace, **(_trace_kwargs or {}),
    )
    out = np.zeros((B, S, D), np.float32)
    for c in range(NCORES):
        out[c // 4] += res.results[c]["y"]
    if _trace:
        return out, res
    return out


if __name__ == "__main__":
    nc = build_nc()
    print("built ok")


# revision 17
# speedup vs baseline: 1.4511x; 1.4511x over previous
"""Trainium2 Bass kernel for nn_Encoder_50852412785097 (sparse_attention).

Math (validated against the jax reference to ~1e-6):
  Per (b, h):
    Q = X wQ_h, K = X wK_h, V = X wV_h              (X = inputs[b], [S, D])
    e = (Q K^T) / sqrt(D)
    x = causal_softmax(e)          # == softmax(e) * tril, renormalized
    rr = den - cumsum(exp(e_row))  # den = masked row sum of exp(e)
    decay = exp((theta^2/den) * (t_j - t_i) * rr)   # == exp(-theta^2 (1-c) dt)
    u = exp(e * decay)             # unnormalized second softmax
    out_h = ((u @ V) / sum_j u) @ wO_h
  out[b] = sum_h out_h

Sharding: 16 (b, h) pairs over 8 cores -> core c handles b = c//4,
heads {2*(c%4), 2*(c%4)+1}. Weights replicated; host sums the 4 partial
outputs per batch.

SPMD program per core: flash-style over 16 row-tiles of 128 rows, only the
causal-active W = 128*(ti+1) columns are computed. cumsum via DVE
tensor_tensor_scan; softmax sums fused into ACT exp via accum_out; AV via
PE 128x128 transposes of u.
"""

import os
import sys

import numpy as np

B, S, H, D = 2, 2048, 8, 64
P = 128
NT = S // P  # 16 row tiles
NH = 2  # heads per core
NCORES = 8
MASK_VAL = -1e30


def _import_concourse():
    try:
        import concourse.bass  # noqa: F401
    except ImportError:
        for p in ("/opt/trn_rl_repo", "/root/.axon_site/_ro/trn_rl_repo"):
            if os.path.isdir(p) and p not in sys.path:
                sys.path.insert(0, p)
        import concourse.bass  # noqa: F401


def build_nc(use_dma_transpose=False, sa_gpsimd=True):
    """Build the SPMD single-core program (same on all 8 cores)."""
    _import_concourse()
    import concourse.bass as bass
    import concourse.bacc as bacc
    from concourse import mybir
    from concourse.tile import TileContext

    f32 = mybir.dt.float32
    bf16 = mybir.dt.bfloat16
    Alu = mybir.AluOpType
    Act = mybir.ActivationFunctionType

    # Bacc (not raw Bass): its finalize() runs move_matmul_waits_to_ldweights
    # + generate_event_semaphores, legalizing the 1-sync-wait-per-instruction
    # hardware constraint that walrus enforces.
    nc = bacc.Bacc("TRN2", target_bir_lowering=False, debug=False)

    # --- external I/O (per core) ---
    xT_h = nc.dram_tensor("xT", [D, S], f32, kind="ExternalInput")     # X^T
    tsj_h = nc.dram_tensor("tsj", [1, S], f32, kind="ExternalInput")   # t_j row
    tsi_h = nc.dram_tensor("tsi", [P, NT], f32, kind="ExternalInput")  # t_i cols
    wq_h = nc.dram_tensor("wq", [D, NH * D], f32, kind="ExternalInput")
    wk_h = nc.dram_tensor("wk", [D, NH * D], f32, kind="ExternalInput")
    wv_h = nc.dram_tensor("wv", [D, NH * D], f32, kind="ExternalInput")
    wo_h = nc.dram_tensor("wo", [D, NH * D], f32, kind="ExternalInput")
    th_h = nc.dram_tensor("th", [1, 1], f32, kind="ExternalInput")
    y_h = nc.dram_tensor("y", [S, D], f32, kind="ExternalOutput")

    # --- NEFF-embedded constants ---
    mask_np = np.triu(np.ones((P, P), np.float32), k=1) * np.float32(MASK_VAL)
    mask_dram = nc.inline_tensor(mask_np, name="maskc")
    ident_dram = nc.inline_tensor(np.eye(P, dtype=np.float32).astype(
        np.dtype("bfloat16") if False else np.float32), name="identc")

    with TileContext(nc) as tc:
        from contextlib import ExitStack

        with ExitStack() as ctx:
            consts = ctx.enter_context(tc.tile_pool(name="consts", bufs=1))

            # DMA inputs into staging tiles, then stage through a single
            # compute engine so downstream consumers wait on ONE semaphore.
            # (walrus allows only one sync-wait on PE LDWEIGHTS: every
            # PE-consumed tile must be DVE-produced; DVE-consumed multi-DMA
            # tiles go through ACT.)
            def load(shape, handle_ap, via, name, dt=f32):
                stage = consts.tile(shape, f32, tag=f"stg_{name}")
                nc.gpsimd.dma_start(out=stage, in_=handle_ap)
                dst = consts.tile(shape, dt, tag=name)
                via(dst, stage)
                return dst

            # PE-consumed: staged via DVE (order: ident/mask first so PE's
            # DVE-tick already covers them by first use)
            mask = load([P, P], mask_dram[:, :], nc.vector.tensor_copy, "mask")
            identb = load([P, P], ident_dram[:, :], nc.vector.tensor_copy,
                          "identb", dt=bf16)
            xT = load([D, S], xT_h[:, :], nc.vector.tensor_copy, "xT")
            xTb = consts.tile([D, S], bf16, tag="xTb")
            nc.vector.tensor_copy(xTb, xT)
            wq = load([D, NH * D], wq_h[:, :], nc.vector.tensor_copy, "wq", dt=bf16)
            wk = load([D, NH * D], wk_h[:, :], nc.vector.tensor_copy, "wk", dt=bf16)
            wv = load([D, NH * D], wv_h[:, :], nc.vector.tensor_copy, "wv", dt=bf16)
            wo = load([D, NH * D], wo_h[:, :], nc.vector.tensor_copy, "wo", dt=bf16)

            # DVE-consumed (ww op): staged via ACT so the ww instruction's
            # waits collapse onto the ACT semaphore
            tsj_ap = tsj_h[:, :]
            tsj_b = bass.AP(
                tensor=tsj_ap.tensor, offset=tsj_ap.offset,
                ap=[[0, P], list(tsj_ap.ap[-1])],
            )
            tsj = load([P, S], tsj_b, nc.scalar.copy, "tsj")
            tsi = load([P, NT], tsi_h[:, :], nc.scalar.copy, "tsi")

            # theta broadcast -> th2 = theta^2 (DVE-consumed once)
            thb = consts.tile([P, 1], f32)
            th_ap = th_h[:, :]
            th_b = bass.AP(
                tensor=th_ap.tensor, offset=th_ap.offset,
                ap=[[0, P], list(th_ap.ap[-1])],
            )
            nc.gpsimd.dma_start(out=thb, in_=th_b)
            th2 = consts.tile([P, 1], f32)
            nc.vector.tensor_mul(th2, thb, thb)

            ones = consts.tile([P, S], f32)
            nc.vector.memset(ones, 1.0)

            # --- projections: qt (scaled by 1/8), kt: [64, NH*S]; v: [128, NH*NT*64] ---
            qt = consts.tile([D, NH * S], bf16)
            kt = consts.tile([D, NH * S], bf16)
            v = consts.tile([P, NH * NT * D], bf16)
            with tc.tile_pool(name="psetup", bufs=2, space="PSUM") as psetup:
                for h in range(NH):
                    for sc in range(S // 512):
                        pq = psetup.tile([D, 512], f32, tag="pq")
                        nc.tensor.matmul(
                            pq, lhsT=wq[:, h * D:(h + 1) * D],
                            rhs=xTb[:, 512 * sc:512 * (sc + 1)],
                            start=True, stop=True,
                        )
                        nc.scalar.mul(qt[:, h * S + 512 * sc: h * S + 512 * (sc + 1)], pq, 0.125)
                        pk = psetup.tile([D, 512], f32, tag="pk")
                        nc.tensor.matmul(
                            pk, lhsT=wk[:, h * D:(h + 1) * D],
                            rhs=xTb[:, 512 * sc:512 * (sc + 1)],
                            start=True, stop=True,
                        )
                        nc.scalar.copy(kt[:, h * S + 512 * sc: h * S + 512 * (sc + 1)], pk)
                    for st in range(NT):
                        pv = psetup.tile([P, D], f32, tag="pv")
                        nc.tensor.matmul(
                            pv, lhsT=xTb[:, P * st:P * (st + 1)],
                            rhs=wv[:, h * D:(h + 1) * D],
                            start=True, stop=True,
                        )
                        nc.scalar.copy(v[:, (h * NT + st) * D:(h * NT + st + 1) * D], pv)

            # --- main pipeline ---
            work = ctx.enter_context(tc.tile_pool(name="work", bufs=3))
            small = ctx.enter_context(tc.tile_pool(name="small", bufs=6))
            ppe = ctx.enter_context(tc.tile_pool(
                name="ppe", bufs=3 if use_dma_transpose else 2, space="PSUM"))
            ppt = (None if use_dma_transpose else ctx.enter_context(
                tc.tile_pool(name="ppt", bufs=2, space="PSUM")))
            pprT = ctx.enter_context(tc.tile_pool(name="pprT", bufs=2, space="PSUM"))
            ppo = ctx.enter_context(tc.tile_pool(name="ppo", bufs=2, space="PSUM"))

            for ti in range(NT):
                W = P * (ti + 1)
                t_i = tsi[:, ti:ti + 1]
                po_h = []
                rden2_h = []
                for h in range(NH):
                    # scores: es = (Q K^T)/8, diag block gets causal mask added
                    es = work.tile([P, S], f32, tag="es")
                    qrow = qt[:, h * S + P * ti: h * S + P * (ti + 1)]
                    j0 = 0
                    while j0 < W:
                        j1 = min(W, j0 + 512)
                        cols = j1 - j0
                        pe = ppe.tile([P, 512], f32, tag="pe")
                        nc.tensor.matmul(
                            pe[:, :cols], lhsT=qrow,
                            rhs=kt[:, h * S + j0: h * S + j1],
                            start=True, stop=True,
                        )
                        if j1 == W:
                            # single-engine readers per PSUM tile (PE WAR
                            # waits must stay on one semaphore)
                            if cols > P:
                                nc.vector.tensor_copy(es[:, j0:j1 - P], pe[:, :cols - P])
                            nc.vector.tensor_add(
                                es[:, W - P:W], pe[:, cols - P:cols], mask)
                        elif (j0 // 512) % 2 == 0:
                            nc.scalar.copy(es[:, j0:j1], pe[:, :cols])
                        else:
                            nc.vector.tensor_copy(es[:, j0:j1], pe[:, :cols])
                        j0 = j1

                    # first softmax pieces: ex1 = exp(es), den = row sum
                    ex1 = work.tile([P, S], f32, tag="ex1")
                    den = small.tile([P, 1], f32, tag="den")
                    nc.scalar.activation(ex1[:, :W], es[:, :W], Act.Exp,
                                         accum_out=den)
                    rden = small.tile([P, 1], f32, tag="rden")
                    nc.vector.reciprocal(rden, den)
                    spp = small.tile([P, 1], f32, tag="spp")
                    nc.vector.tensor_mul(spp, th2, rden)

                    # rr = den - cumsum(ex1)  (scan: state=(1*state)-ex1_t)
                    rr = work.tile([P, S], f32, tag="rr")
                    nc.vector.tensor_tensor_scan(
                        rr[:, :W], ones[:, :W], ex1[:, :W], initial=den,
                        op0=Alu.mult, op1=Alu.subtract,
                    )
                    # ww = (t_j - t_i) * rr   (in-place into rr)
                    nc.vector.scalar_tensor_tensor(
                        rr[:, :W], in0=tsj[:, :W], scalar=t_i, in1=rr[:, :W],
                        op0=Alu.subtract, op1=Alu.mult,
                    )
                    # decay = exp(spp * ww); sarr = es * decay (in-place);
                    # u = exp(sarr), den2 = row sum
                    e2 = work.tile([P, S], f32, tag="e2")
                    nc.scalar.activation(e2[:, :W], rr[:, :W], Act.Exp,
                                         scale=spp)
                    (nc.gpsimd if sa_gpsimd else nc.vector).tensor_mul(
                        e2[:, :W], es[:, :W], e2[:, :W])
                    u = work.tile([P, S], bf16, tag="u")
                    den2 = small.tile([P, 1], f32, tag="den2")
                    nc.scalar.activation(u[:, :W], e2[:, :W], Act.Exp,
                                         accum_out=den2)
                    rden2 = small.tile([P, 1], f32, tag="rden2")
                    nc.vector.reciprocal(rden2, den2)
                    rden2_h.append(rden2)

                    # AV: retT[e, i] = sum_j v[j, e] u[i, j]
                    prT = pprT.tile([D, P], f32, tag="prT")
                    njb = ti + 1
                    for g0 in range(0, njb, 4):
                        gn = min(4, njb - g0)
                        uT4 = small.tile([P, 4 * P], bf16, tag="uT4")
                        if use_dma_transpose:
                            for q in range(gn):
                                nc.sync.dma_start_transpose(
                                    uT4[:, q * P:(q + 1) * P],
                                    u[:, (g0 + q) * P:(g0 + q + 1) * P])
                        else:
                            pt = ppt.tile([P, 4 * P], bf16, tag="pt")
                            for q in range(gn):
                                nc.tensor.transpose(
                                    pt[:, q * P:(q + 1) * P],
                                    u[:, (g0 + q) * P:(g0 + q + 1) * P], identb)
                            nc.scalar.copy(uT4[:, :gn * P], pt[:, :gn * P])
                        for q in range(gn):
                            jb = g0 + q
                            nc.tensor.matmul(
                                prT, lhsT=v[:, (h * NT + jb) * D:(h * NT + jb + 1) * D],
                                rhs=uT4[:, q * P:(q + 1) * P],
                                start=(jb == 0), stop=(jb == ti),
                            )
                    rT = small.tile([D, P], bf16, tag="rT")
                    nc.scalar.copy(rT, prT)
                    po = ppo.tile([P, D], f32, tag="po")
                    nc.tensor.matmul(po, lhsT=rT, rhs=wo[:, h * D:(h + 1) * D],
                                     start=True, stop=True)
                    po_h.append(po)

                # y = po0/den2_0 + po1/den2_1 ; DMA out
                t0 = small.tile([P, D], f32, tag="t0")
                nc.vector.tensor_scalar(t0, po_h[0], scalar1=rden2_h[0],
                                        scalar2=None, op0=Alu.mult)
                ys = small.tile([P, D], f32, tag="ys")
                nc.vector.scalar_tensor_tensor(
                    ys, in0=po_h[1], scalar=rden2_h[1], in1=t0,
                    op0=Alu.mult, op1=Alu.add,
                )
                nc.sync.dma_start(out=y_h[P * ti:P * (ti + 1), :], in_=ys)

    if not nc.is_finalized():
        nc.finalize()
    return nc


_NC_CACHE = {}

KERNEL_FLAGS = {}


def _get_nc():
    key = tuple(sorted(KERNEL_FLAGS.items()))
    if key not in _NC_CACHE:
        _NC_CACHE[key] = build_nc(**KERNEL_FLAGS)
    return _NC_CACHE[key]


def make_in_maps(inputs, timestamp, wQ, wK, wV, wO, theta):
    x = np.asarray(inputs, np.float32)
    t = np.asarray(timestamp).astype(np.float32)
    wQ = np.asarray(wQ, np.float32)
    wK = np.asarray(wK, np.float32)
    wV = np.asarray(wV, np.float32)
    wO = np.asarray(wO, np.float32)
    theta = np.asarray(theta, np.float32)

    in_maps = []
    for c in range(NCORES):
        b = c // 4
        h0 = NH * (c % 4)
        in_maps.append({
            "xT": np.ascontiguousarray(x[b].T),
            "tsj": np.ascontiguousarray(t[b][None, :]),
            "tsi": np.ascontiguousarray(t[b].reshape(NT, P).T),
            "wq": np.ascontiguousarray(np.concatenate([wQ[h0], wQ[h0 + 1]], axis=1)),
            "wk": np.ascontiguousarray(np.concatenate([wK[h0], wK[h0 + 1]], axis=1)),
            "wv": np.ascontiguousarray(np.concatenate([wV[h0], wV[h0 + 1]], axis=1)),
            "wo": np.ascontiguousarray(np.concatenate(
                [wO[h0 * D:(h0 + 1) * D], wO[(h0 + 1) * D:(h0 + 2) * D]], axis=1)),
            "th": np.ascontiguousarray(theta.reshape(1, 1)),
        })
    return in_maps


def kernel(inputs, timestamp, wQ, wK, wV, wO, theta, _trace=False, _trace_kwargs=None):
    _import_concourse()
    from concourse.bass_utils import run_bass_kernel_spmd

    nc = _get_nc()
    in_maps = make_in_maps(inputs, timestamp, wQ, wK, wV, wO, theta)
    res = run_bass_kernel_spmd(
        nc, in_maps, list(range(NCORES)),
        trace=_trace, **(_trace_kwargs or {}),
    )
    out = np.zeros((B, S, D), np.float32)
    for c in range(NCORES):
        out[c // 4] += res.results[c]["y"]
    if _trace:
        return out, res
    return out


if __name__ == "__main__":
    nc = build_nc()
    print("built ok")
